# revision 1
# baseline (speedup 1.0000x reference)
"""Trainium2 Bass kernel for nn_ChannelCompressAttention.

Shapes: x (8, 4096, 1024) f32, w_qkv (3072, 1024) f32, w_conv1 (1024,) f32.
Output: (8, 4096, 1024) f32.

Math: with q,k,v = split(x @ w_qkv^T), agent = q @ w_conv1,
  aa   = softmax_c(scale * agent @ k)          # (c,)
  p    = softmax_n(aa @ v^T)                   # (n,)
  out  = softmax(agent[:,:,None], -1) * (p @ v)[None]
The last softmax is over a singleton axis == all-ones, so every output row
equals agent_v = p @ v, and all q/k/v uses are rank-1 contractions.  The
3c x c projection is therefore never materialized:
  u  = scale * Wq^T w_conv1      agent = x u           (per batch)
  s  = x^T agent                 z     = Wk s
  aa = softmax(z)                t     = Wv^T aa
  sc = x t                       p     = softmax(sc)
  r  = x^T p                     out_row = Wv r
~206 GFLOP collapses to ~0.5 GFLOP; the kernel is HBM-bound.

Sharding: data-parallel over batch, one batch per NeuronCore (8 cores).

On-core mapping (x resident in SBUF, natural (n-part, c-free) layout):
  - n-contractions (s, r, u, t): TensorE row-form: lhsT = per-tile
    n/d-vector (128,1) (1-column weight load), rhs = tile (128, 512) fp32
    streaming, accumulating into (1,512) PSUM pairs.  (The N=1 column
    form is worse: fp32 LDWEIGHTS of a 128x128 block costs ~333ns.)
  - c-contractions (agent, sc, z, out_row): VectorE fused
    scalar_tensor_tensor (multiply + free-dim sum in one instruction).
    (tensor_tensor_reduce faults the DVE on this HW - do not use.)
  - softmax partition sums: TensorE matmul against a ones vector; exp on
    ScalarE (no max subtraction needed: logits are O(10) here, fp32-safe).
  - second-softmax normalization is deferred: r accumulates unnormalized
    exp(sc) per tile so PE overlaps the score pass; 1/Z2 is folded into
    the ACT copies of r's partition-broadcast.
Wk/Wv rows are loaded interleaved (row 8p+j -> partition p, tile j) so
the final (128,8) result column flat-DMAs into a c-ordered row.
"""

import sys

for _p in ("/opt/trn_rl_repo", "/opt/pypackages"):
    if _p not in sys.path:
        sys.path.insert(0, _p)

import numpy as np

import concourse.bacc as bacc
import concourse.mybir as mybir
import concourse.tile as tile
from concourse.bass_utils import run_bass_kernel_spmd

B, N, C = 8, 4096, 1024
P = 128
NT = N // P          # 32 x-tiles per batch
J = C // P           # 8 weight tiles per matrix
F32 = mybir.dt.float32
SCALE = float(C) ** -0.5
H = 512


def _build():
    # Bacc (not raw Bass): its compile() splits multi-wait instructions into
    # event semaphores (TRN2 codegen allows 1 wait/inst) and allocates regs.
    nc = bacc.Bacc(None)
    xb = nc.declare_dram_parameter("xb", [N, C], F32, isOutput=False)
    wqkv = nc.declare_dram_parameter("w_qkv", [3 * C, C], F32, isOutput=False)
    wc = nc.declare_dram_parameter("w_conv1", [C], F32, isOutput=False)
    out = nc.declare_dram_parameter("out", [N, C], F32, isOutput=True)

    mult = mybir.AluOpType.mult
    add = mybir.AluOpType.add
    AF = mybir.ActivationFunctionType

    with tile.TileContext(nc) as tc:
        with (
            tc.tile_pool(name="xres", bufs=NT) as xpool,
            tc.tile_pool(name="wst", bufs=3) as wst,
            tc.tile_pool(name="wv", bufs=J) as wvpool,
            tc.tile_pool(name="bc", bufs=2) as bcpool,
            tc.tile_pool(name="scr", bufs=2) as scrpool,
            tc.tile_pool(name="vec", bufs=6) as vecpool,
            tc.tile_pool(name="rows", bufs=2) as rows,
            tc.tile_pool(name="small", bufs=1) as small,
            tc.tile_pool(name="ps", bufs=8, space="PSUM") as psp,
        ):
            ones_m = small.tile([1, P], F32, tag="ones_m")   # lhsT: row bcast
            nc.vector.memset(ones_m, 1.0)
            ones_k = small.tile([P, 1], F32, tag="ones_k")   # rhs: part sum
            nc.vector.memset(ones_k, 1.0)

            F32R = mybir.dt.float32r

            def r_(ap):
                # fp32 matmul streams at 4 cycles/row; float32r at 1 (N>=256).
                return ap.bitcast(F32R)

            def acc_pair(nm):
                lo = psp.tile([1, H], F32, tag="ps", name=f"{nm}_lo")
                hi = psp.tile([1, H], F32, tag="ps", name=f"{nm}_hi")
                return lo, hi

            def psum_to_row(ps_lo, ps_hi, scale=1.0):
                row = rows.tile([1, C], F32, tag="row")
                nc.scalar.activation(out=row[:, 0:H], in_=ps_lo, func=AF.Copy,
                                     scale=scale)
                nc.scalar.activation(out=row[:, H:C], in_=ps_hi, func=AF.Copy,
                                     scale=scale)
                return row

            def bcast_row(row, scale=1.0):
                dest = bcpool.tile([P, C], F32, tag="bc")
                for h in range(2):
                    ps = psp.tile([P, H], F32, tag="ps")
                    nc.tensor.matmul(ps, lhsT=ones_m,
                                     rhs=row[:, h * H:(h + 1) * H],
                                     start=True, stop=True)
                    nc.scalar.activation(out=dest[:, h * H:(h + 1) * H],
                                         in_=ps, func=AF.Copy, scale=scale)
                return dest

            # w_conv1 as (128, 8): column j = contiguous d-chunk j
            wc_sb = small.tile([P, J], F32, tag="wc")
            nc.gpsimd.dma_start(out=r_(wc_sb),
                                in_=r_(wc.rearrange("(j p) -> p j", p=P)))

            # ---- u = scale * Wq^T w_conv1 ----
            u_lo, u_hi = acc_pair("u")
            for j in range(J):
                wq_j = wst.tile([P, C], F32, tag="w")
                nc.sync.dma_start(out=r_(wq_j),
                                  in_=r_(wqkv[j * P:(j + 1) * P, :]))
                nc.tensor.matmul(u_lo, lhsT=r_(wc_sb[:, j:j + 1]),
                                 rhs=r_(wq_j[:, 0:H]),
                                 start=(j == 0), stop=(j == J - 1))
                nc.tensor.matmul(u_hi, lhsT=r_(wc_sb[:, j:j + 1]),
                                 rhs=r_(wq_j[:, H:C]),
                                 start=(j == 0), stop=(j == J - 1))
            u_bc = bcast_row(psum_to_row(u_lo, u_hi, scale=SCALE))

            # ---- stream x; agent_i = x_i u (DVE), s += x_i^T agent_i (PE) ----
            x_tiles = []
            s_lo, s_hi = acc_pair("s")
            for i in range(NT):
                xt = xpool.tile([P, C], F32, tag="x")
                nc.sync.dma_start(out=r_(xt), in_=r_(xb[i * P:(i + 1) * P, :]))
                x_tiles.append(xt)
                agent_i = vecpool.tile([P, 1], F32, tag="agent")
                scr = scrpool.tile([P, C], F32, tag="scr")
                nc.vector.scalar_tensor_tensor(
                    out=scr, in0=xt, scalar=1.0, in1=u_bc,
                    op0=mult, op1=mult, accum_out=r_(agent_i))
                nc.tensor.matmul(s_lo, lhsT=r_(agent_i), rhs=r_(xt[:, 0:H]),
                                 start=(i == 0), stop=(i == NT - 1))
                nc.tensor.matmul(s_hi, lhsT=r_(agent_i), rhs=r_(xt[:, H:C]),
                                 start=(i == 0), stop=(i == NT - 1))
            s_bc = bcast_row(psum_to_row(s_lo, s_hi))

            # ---- z = Wk s (rows interleaved: tile j partition p = row 8p+j) ----
            wkb = wqkv[C:2 * C, :].rearrange("(p j) c -> j p c", j=J)
            z_col = small.tile([P, J], F32, tag="z")
            for j in range(J):
                wk_j = wst.tile([P, C], F32, tag="w")
                nc.sync.dma_start(out=wk_j, in_=wkb[j])
                scr = scrpool.tile([P, C], F32, tag="scr")
                nc.vector.scalar_tensor_tensor(
                    out=scr, in0=wk_j, scalar=1.0, in1=s_bc,
                    op0=mult, op1=mult, accum_out=z_col[:, j:j + 1])

            # ---- softmax over c ----
            ez = small.tile([P, J], F32, tag="ez")
            ez_sum = small.tile([P, 1], F32, tag="ezs")
            nc.scalar.activation(out=r_(ez), in_=z_col, func=AF.Exp,
                                 accum_out=ez_sum)
            z1 = psp.tile([1, 1], F32, tag="ps")
            nc.tensor.matmul(z1, lhsT=ez_sum, rhs=ones_k, start=True, stop=True)
            rz1 = small.tile([1, 1], F32, tag="rz1")
            nc.vector.reciprocal(out=rz1, in_=z1)

            # ---- t = Wv^T ez / Z1 (Wv resident, rows interleaved) ----
            wvb = wqkv[2 * C:3 * C, :].rearrange("(p j) c -> j p c", j=J)
            wv_tiles = []
            t_lo, t_hi = acc_pair("t")
            for j in range(J):
                wv_j = wvpool.tile([P, C], F32, tag="wv")
                nc.sync.dma_start(out=r_(wv_j), in_=r_(wvb[j]))
                wv_tiles.append(wv_j)
                nc.tensor.matmul(t_lo, lhsT=r_(ez[:, j:j + 1]),
                                 rhs=r_(wv_j[:, 0:H]),
                                 start=(j == 0), stop=(j == J - 1))
                nc.tensor.matmul(t_hi, lhsT=r_(ez[:, j:j + 1]),
                                 rhs=r_(wv_j[:, H:C]),
                                 start=(j == 0), stop=(j == J - 1))
            t_bc = bcast_row(psum_to_row(t_lo, t_hi, scale=rz1))

            # ---- sc_i = x_i t (DVE); ep_i = exp(sc_i) (ACT);
            #      r += x_i^T ep_i (PE, unnormalized) ----
            ep_col = small.tile([P, NT], F32, tag="epc")
            r_lo, r_hi = acc_pair("r")
            for i in range(NT):
                xt = x_tiles[i]
                sc_i = vecpool.tile([P, 1], F32, tag="sc")
                scr = scrpool.tile([P, C], F32, tag="scr")
                nc.vector.scalar_tensor_tensor(
                    out=scr, in0=xt, scalar=1.0, in1=t_bc,
                    op0=mult, op1=mult, accum_out=sc_i)
                nc.scalar.activation(out=r_(ep_col[:, i:i + 1]), in_=sc_i,
                                     func=AF.Exp)
                nc.tensor.matmul(r_lo, lhsT=r_(ep_col[:, i:i + 1]),
                                 rhs=r_(xt[:, 0:H]),
                                 start=(i == 0), stop=(i == NT - 1))
                nc.tensor.matmul(r_hi, lhsT=r_(ep_col[:, i:i + 1]),
                                 rhs=r_(xt[:, H:C]),
                                 start=(i == 0), stop=(i == NT - 1))
            # Z2 = sum(ep); fold 1/Z2 into r's broadcast copies
            ep_rs = small.tile([P, 1], F32, tag="eprs")
            nc.vector.tensor_reduce(out=ep_rs, in_=ep_col,
                                    axis=mybir.AxisListType.X, op=add)
            z2 = psp.tile([1, 1], F32, tag="ps")
            nc.tensor.matmul(z2, lhsT=ep_rs, rhs=ones_k, start=True, stop=True)
            rz2 = small.tile([1, 1], F32, tag="rz2")
            nc.vector.reciprocal(out=rz2, in_=z2)
            rz2_bc = small.tile([P, 1], F32, tag="rz2b")
            nc.gpsimd.partition_broadcast(rz2_bc, rz2)
            r_bc = bcast_row(psum_to_row(r_lo, r_hi), scale=rz2_bc)

            # ---- out_row[8p+j] = (Wv r)[8p+j] ----
            vo_col = small.tile([P, J], F32, tag="vo")
            for j in range(J):
                scr = scrpool.tile([P, C], F32, tag="scr")
                nc.vector.scalar_tensor_tensor(
                    out=scr, in0=wv_tiles[j], scalar=1.0, in1=r_bc,
                    op0=mult, op1=mult, accum_out=vo_col[:, j:j + 1])
            vo_row = rows.tile([1, C], F32, tag="row")
            nc.sync.dma_start(out=vo_row, in_=vo_col)
            ob = bcast_row(vo_row)
            for o in range(NT):
                nc.sync.dma_start(out=out[o * P:(o + 1) * P, :], in_=ob)

    return nc


_CACHE = {}


def _get_nc():
    if "nc" not in _CACHE:
        nc = _build()
        nc.finalize()
        _CACHE["nc"] = nc
    return _CACHE["nc"]


def _in_maps(x, w_qkv, w_conv1):
    return [{"xb": x[b], "w_qkv": w_qkv, "w_conv1": w_conv1} for b in range(B)]


def run(x, w_qkv, w_conv1, **spmd_kwargs):
    x = np.ascontiguousarray(np.asarray(x, dtype=np.float32))
    w_qkv = np.ascontiguousarray(np.asarray(w_qkv, dtype=np.float32))
    w_conv1 = np.ascontiguousarray(np.asarray(w_conv1, dtype=np.float32))
    res = run_bass_kernel_spmd(_get_nc(), _in_maps(x, w_qkv, w_conv1),
                               list(range(B)), **spmd_kwargs)
    out = np.stack([res.results[b]["out"] for b in range(B)], axis=0)
    return out, res


def kernel(x, w_qkv, w_conv1):
    out, _ = run(x, w_qkv, w_conv1)
    return out



# revision 5
# speedup vs baseline: 1.6203x; 1.6203x over previous
"""Trainium2 Bass kernel for nn_ChannelCompressAttention.

Shapes: x (8, 4096, 1024) f32, w_qkv (3072, 1024) f32, w_conv1 (1024,) f32.
Output: (8, 4096, 1024) f32.

Math: with q,k,v = split(x @ w_qkv^T), agent = q @ w_conv1,
  aa   = softmax_c(scale * agent @ k)          # (c,)
  p    = softmax_n(aa @ v^T)                   # (n,)
  out  = softmax(agent[:,:,None], -1) * (p @ v)[None]
The last softmax is over a singleton axis == all-ones, so every output row
equals agent_v = p @ v, and all q/k/v uses are rank-1 contractions.  The
3c x c projection is therefore never materialized:
  u  = scale * Wq^T w_conv1      agent = x u           (per batch)
  s  = x^T agent                 z     = Wk s
  aa = softmax(z)                t     = Wv^T aa
  sc = x t                       p     = softmax(sc)
  r  = x^T p                     out_row = Wv r
~206 GFLOP collapses to ~0.5 GFLOP; the kernel is HBM/DVE-bound.

Sharding: data-parallel over batch, one batch per NeuronCore (8 cores).

This version vs the fp32 original (225 us):
  - inputs are cast to bf16 on the host: halves HBM traffic (14 MiB/core
    in) and doubles DVE throughput (TT-class ops hit 2x mode).  Simulated
    end-to-end rel_l2 vs fp64 reference: 8.9e-3 (gate 2e-2).
  - big DMAs: x in 8x1MiB chunk tiles, each weight matrix in one 2 MiB
    transfer (interleaved row layout baked into the access pattern), vs
    ~80 x 512 KiB transfers before.
  - every output row is identical (the softmax over the singleton axis is
    all-ones), so the device emits only the (1024,) row per batch and the
    host broadcasts to (4096, 1024): kills the 16 MiB/core output write.
On-core mapping (x resident in SBUF, (n-part, c-free) layout):
  - n-contractions (s, r, u, t): TensorE rank-1 row-form, accumulating
    into (1,512) PSUM pairs.
  - c-contractions (agent, sc, z, out_row): VectorE scalar_tensor_tensor
    (multiply + free-dim sum in one instruction), bf16 2x mode.
  - softmax partition sums via TensorE matmul against ones; exp on
    ScalarE (logits are O(30), fp32-safe without max subtraction).
  - second-softmax normalization deferred: 1/Z2 folded into the ACT
    copies of r's partition-broadcast.
Wk/Wv rows are interleaved (row 8p+j -> partition p, tile j) so the final
(128,8) result column flat-DMAs into a c-ordered row.
"""

import sys

for _p in ("/opt/trn_rl_repo", "/opt/pypackages"):
    if _p not in sys.path:
        sys.path.insert(0, _p)

import ml_dtypes
import numpy as np

import concourse.bacc as bacc
import concourse.mybir as mybir
import concourse.tile as tile
from concourse.bass_utils import run_bass_kernel_spmd

B, N, C = 8, 4096, 1024
P = 128
NT = N // P          # 32 x-tiles per batch
J = C // P           # 8 weight tiles per matrix
TPC = 4              # x-tiles per DMA chunk
NCH = NT // TPC      # 8 chunks
F32 = mybir.dt.float32
BF16 = mybir.dt.bfloat16
SCALE = float(C) ** -0.5
H = 512


def _build():
    nc = bacc.Bacc(None)
    xb = nc.declare_dram_parameter("xb", [N, C], BF16, isOutput=False)
    wqkv = nc.declare_dram_parameter("w_qkv", [3 * C, C], BF16, isOutput=False)
    wc = nc.declare_dram_parameter("w_conv1", [C], BF16, isOutput=False)
    out = nc.declare_dram_parameter("out", [C], F32, isOutput=True)

    mult = mybir.AluOpType.mult
    add = mybir.AluOpType.add
    AF = mybir.ActivationFunctionType
    F32R = mybir.dt.float32r

    def r_(ap):
        return ap.bitcast(F32R)

    with tile.TileContext(nc) as tc:
        with (
            tc.tile_pool(name="xres", bufs=NCH) as xpool,
            tc.tile_pool(name="wq", bufs=1) as wqpool,
            tc.tile_pool(name="wk", bufs=1) as wkpool,
            tc.tile_pool(name="wv", bufs=1) as wvpool,
            tc.tile_pool(name="bc", bufs=4) as bcpool,
            tc.tile_pool(name="scr", bufs=2) as scrpool,
            tc.tile_pool(name="vec", bufs=8) as vecpool,
            tc.tile_pool(name="rows", bufs=2) as rows,
            tc.tile_pool(name="small", bufs=1) as small,
            tc.tile_pool(name="ps", bufs=8, space="PSUM") as psp,
        ):
            ones_m = small.tile([1, P], BF16, tag="ones_m")   # lhsT: row bcast
            nc.vector.memset(ones_m, 1.0)
            ones_k = small.tile([P, 1], F32, tag="ones_k")    # rhs: part sum
            nc.vector.memset(ones_k, 1.0)

            # ---- all input DMAs up front (HWDGE FIFO drains in order) ----
            wc_sb = small.tile([P, J], BF16, tag="wc")        # [p,j]=wc[j*128+p]
            nc.sync.dma_start(out=wc_sb, in_=wc.rearrange("(j p) -> p j", p=P))
            wq_t = wqpool.tile([P, J * C], BF16, tag="wq")    # blk j = rows j*128+p
            nc.sync.dma_start(out=wq_t.rearrange("p (j c) -> p j c", j=J),
                              in_=wqkv[0:C, :].rearrange("(j p) c -> p j c", p=P))
            x_chunks = []
            for g in range(NCH):
                xg = xpool.tile([P, TPC * C], BF16, tag="x")  # blk k = tile g*4+k
                nc.sync.dma_start(
                    out=xg.rearrange("p (t c) -> p t c", t=TPC),
                    in_=xb[g * TPC * P:(g + 1) * TPC * P, :].rearrange(
                        "(t p) c -> p t c", p=P))
                x_chunks.append(xg)
            wk_t = wkpool.tile([P, J * C], BF16, tag="wk")    # blk j = rows p*8+j
            nc.sync.dma_start(out=wk_t.rearrange("p (j c) -> p j c", j=J),
                              in_=wqkv[C:2 * C, :].rearrange("(p j) c -> p j c", j=J))
            wv_t = wvpool.tile([P, J * C], BF16, tag="wv")    # blk j = rows p*8+j
            nc.sync.dma_start(out=wv_t.rearrange("p (j c) -> p j c", j=J),
                              in_=wqkv[2 * C:3 * C, :].rearrange("(p j) c -> p j c", j=J))

            def acc_pair(nm):
                lo = psp.tile([1, H], F32, tag="ps", name=f"{nm}_lo")
                hi = psp.tile([1, H], F32, tag="ps", name=f"{nm}_hi")
                return lo, hi

            def psum_to_row(ps_lo, ps_hi, scale=1.0):
                row = rows.tile([1, C], BF16, tag="row")
                nc.scalar.activation(out=row[:, 0:H], in_=ps_lo, func=AF.Copy,
                                     scale=scale)
                nc.scalar.activation(out=row[:, H:C], in_=ps_hi, func=AF.Copy,
                                     scale=scale)
                return row

            def bcast_row(row, scale=1.0):
                dest = bcpool.tile([P, C], BF16, tag="bc")
                for h in range(2):
                    ps = psp.tile([P, H], F32, tag="ps")
                    nc.tensor.matmul(ps, lhsT=ones_m,
                                     rhs=row[:, h * H:(h + 1) * H],
                                     start=True, stop=True)
                    nc.scalar.activation(out=dest[:, h * H:(h + 1) * H],
                                         in_=ps, func=AF.Copy, scale=scale)
                return dest

            # ---- u = scale * Wq^T w_conv1 ----
            u_lo, u_hi = acc_pair("u")
            for j in range(J):
                nc.tensor.matmul(u_lo, lhsT=wc_sb[:, j:j + 1],
                                 rhs=wq_t[:, j * C:j * C + H],
                                 start=(j == 0), stop=(j == J - 1))
                nc.tensor.matmul(u_hi, lhsT=wc_sb[:, j:j + 1],
                                 rhs=wq_t[:, j * C + H:(j + 1) * C],
                                 start=(j == 0), stop=(j == J - 1))
            u_bc = bcast_row(psum_to_row(u_lo, u_hi, scale=SCALE))

            # ---- pass 1: agent_i = x_i u (DVE), s += x_i^T agent_i (PE) ----
            s_lo, s_hi = acc_pair("s")
            for i in range(NT):
                xt = x_chunks[i // TPC][:, (i % TPC) * C:(i % TPC + 1) * C]
                agent_f = vecpool.tile([P, 1], F32, tag="agf")
                scr = scrpool.tile([P, C], BF16, tag="scr")
                nc.vector.scalar_tensor_tensor(
                    out=scr, in0=xt, scalar=1.0, in1=u_bc,
                    op0=mult, op1=mult, accum_out=agent_f)
                agent_b = vecpool.tile([P, 1], BF16, tag="agb")
                nc.scalar.activation(out=agent_b, in_=agent_f, func=AF.Copy)
                nc.tensor.matmul(s_lo, lhsT=agent_b, rhs=xt[:, 0:H],
                                 start=(i == 0), stop=(i == NT - 1))
                nc.tensor.matmul(s_hi, lhsT=agent_b, rhs=xt[:, H:C],
                                 start=(i == 0), stop=(i == NT - 1))
            s_bc = bcast_row(psum_to_row(s_lo, s_hi))

            # ---- z[p*8+j] = Wk[p*8+j] . s (DVE) ----
            z_col = small.tile([P, J], F32, tag="z")
            for j in range(J):
                scr = scrpool.tile([P, C], BF16, tag="scr")
                nc.vector.scalar_tensor_tensor(
                    out=scr, in0=wk_t[:, j * C:(j + 1) * C], scalar=1.0,
                    in1=s_bc, op0=mult, op1=mult, accum_out=z_col[:, j:j + 1])

            # ---- softmax over c (no max-sub: |z| < 40, fp32-safe) ----
            ez = small.tile([P, J], BF16, tag="ez")
            ez_sum = small.tile([P, 1], F32, tag="ezs")
            nc.scalar.activation(out=ez, in_=z_col, func=AF.Exp,
                                 accum_out=ez_sum)
            z1 = psp.tile([1, 1], F32, tag="ps")
            nc.tensor.matmul(z1, lhsT=ez_sum, rhs=ones_k,
                             start=True, stop=True)
            rz1 = small.tile([1, 1], F32, tag="rz1")
            nc.vector.reciprocal(out=rz1, in_=z1)

            # ---- t = Wv^T ez / Z1 ----
            t_lo, t_hi = acc_pair("t")
            for j in range(J):
                nc.tensor.matmul(t_lo, lhsT=ez[:, j:j + 1],
                                 rhs=wv_t[:, j * C:j * C + H],
                                 start=(j == 0), stop=(j == J - 1))
                nc.tensor.matmul(t_hi, lhsT=ez[:, j:j + 1],
                                 rhs=wv_t[:, j * C + H:(j + 1) * C],
                                 start=(j == 0), stop=(j == J - 1))
            t_bc = bcast_row(psum_to_row(t_lo, t_hi, scale=rz1))

            # ---- pass 2: sc_i = x_i t (DVE); ep_i = exp(sc_i) (ACT);
            #      r += x_i^T ep_i (PE, unnormalized) ----
            ep_col = small.tile([P, NT], BF16, tag="epc")
            r_lo, r_hi = acc_pair("r")
            for i in range(NT):
                xt = x_chunks[i // TPC][:, (i % TPC) * C:(i % TPC + 1) * C]
                sc_f = vecpool.tile([P, 1], F32, tag="sc")
                scr = scrpool.tile([P, C], BF16, tag="scr")
                nc.vector.scalar_tensor_tensor(
                    out=scr, in0=xt, scalar=1.0, in1=t_bc,
                    op0=mult, op1=mult, accum_out=sc_f)
                nc.scalar.activation(out=ep_col[:, i:i + 1], in_=sc_f,
                                     func=AF.Exp)
                nc.tensor.matmul(r_lo, lhsT=ep_col[:, i:i + 1], rhs=xt[:, 0:H],
                                 start=(i == 0), stop=(i == NT - 1))
                nc.tensor.matmul(r_hi, lhsT=ep_col[:, i:i + 1], rhs=xt[:, H:C],
                                 start=(i == 0), stop=(i == NT - 1))
            # Z2 = sum(ep); fold 1/Z2 into r's broadcast copies
            ep_rs = small.tile([P, 1], F32, tag="eprs")
            nc.vector.tensor_reduce(out=ep_rs, in_=ep_col,
                                    axis=mybir.AxisListType.X, op=add)
            z2 = psp.tile([1, 1], F32, tag="ps")
            nc.tensor.matmul(z2, lhsT=ep_rs, rhs=ones_k,
                             start=True, stop=True)
            rz2 = small.tile([1, 1], F32, tag="rz2")
            nc.vector.reciprocal(out=rz2, in_=z2)
            rz2_bc = small.tile([P, 1], F32, tag="rz2b")
            nc.gpsimd.partition_broadcast(rz2_bc, rz2)
            r_bc = bcast_row(psum_to_row(r_lo, r_hi), scale=rz2_bc)

            # ---- out_row[p*8+j] = Wv[p*8+j] . r ----
            vo_col = small.tile([P, J], F32, tag="vo")
            for j in range(J):
                scr = scrpool.tile([P, C], BF16, tag="scr")
                nc.vector.scalar_tensor_tensor(
                    out=scr, in0=wv_t[:, j * C:(j + 1) * C], scalar=1.0,
                    in1=r_bc, op0=mult, op1=mult, accum_out=vo_col[:, j:j + 1])
            nc.sync.dma_start(out=out.rearrange("(p j) -> p j", p=P), in_=vo_col)

    return nc


_CACHE = {}


def _get_nc():
    if "nc" not in _CACHE:
        nc = _build()
        nc.finalize()
        _CACHE["nc"] = nc
    return _CACHE["nc"]


def _in_maps(x, w_qkv, w_conv1):
    xb = x.astype(ml_dtypes.bfloat16)
    wb = w_qkv.astype(ml_dtypes.bfloat16)
    cb = w_conv1.astype(ml_dtypes.bfloat16)
    return [{"xb": np.ascontiguousarray(xb[b]), "w_qkv": wb, "w_conv1": cb}
            for b in range(B)]


def run(x, w_qkv, w_conv1, **spmd_kwargs):
    x = np.asarray(x, dtype=np.float32)
    w_qkv = np.asarray(w_qkv, dtype=np.float32)
    w_conv1 = np.asarray(w_conv1, dtype=np.float32)
    res = run_bass_kernel_spmd(_get_nc(), _in_maps(x, w_qkv, w_conv1),
                               list(range(B)), **spmd_kwargs)
    av = np.stack([res.results[b]["out"] for b in range(B)], axis=0)  # (B, C)
    # every output row equals agent_v (softmax over singleton axis == 1)
    out = np.ascontiguousarray(
        np.broadcast_to(av[:, None, :], (B, N, C)), dtype=np.float32)
    return out, res


def kernel(x, w_qkv, w_conv1):
    out, _ = run(x, w_qkv, w_conv1)
    return out
